# revision 2
# baseline (speedup 1.0000x reference)
"""Bahdanau attention TRN2 Bass kernel.

kernel(**inputs) takes the FULL inputs (as produced by setup_inputs()):
    dec_hidden [32, 1024] f32, enc_outputs [32, 2048, 2048] f32,
    W_s [1024, 1024] f32, W_h [1024, 2048] f32, v [1024] f32
and returns (ctx [32, 2048] f32, attn [32, 2048] f32), matching

    s      = dec_hidden @ W_s.T
    h      = enc_outputs @ W_h.T
    scores = einsum('bld,d->bl', tanh(s[:,None,:] + h), v)
    attn   = softmax(scores, axis=1)
    ctx    = einsum('bl,ble->be', attn, enc_outputs)

Distribution: data-parallel over batch, 4 batch elements per NeuronCore on
8 cores, no collectives.  On-core layout: h is produced transposed
(hT[j, l], j on partitions) so the s-projection folds into the ACT
engine's per-partition bias and tanh(h + s) is a single ACT pass over the
matmul PSUM output; enc tiles are transposed on-chip with PE transposes;
the big matmuls run as float32r (full PE rate at N>=256).
"""

import json as _json
from contextlib import ExitStack

import numpy as np

_B, _L, _D, _E = 32, 2048, 1024, 2048
_NCORES = 8


# ----------------------------------------------------------------------------
# Workaround: this walrus build rejects instructions carrying more than one
# semaphore wait ("Too many sync wait commands").  Split extra waits onto
# preceding same-engine NoOps at BIR-serialization time.
# ----------------------------------------------------------------------------
_ws_counter = [0]


def _split_instruction_waits(inst, max_waits=1):
    waits = inst.get("sync_info", {}).get("on_wait") or []
    if len(waits) <= max_waits:
        return [inst]
    out = []
    extra = waits[:-max_waits]
    inst["sync_info"]["on_wait"] = waits[-max_waits:]
    for i in range(0, len(extra), max_waits):
        _ws_counter[0] += 1
        out.append({
            "debug": inst.get("debug", 0),
            "engine": inst["engine"],
            "ins": [],
            "name": f"I-ws{_ws_counter[0]}",
            "opcode": "NoOp",
            "outs": [],
            "sync_info": {"on_update": [], "on_wait": extra[i:i + max_waits]},
        })
    out.append(inst)
    return out


def _walk_split(obj):
    if isinstance(obj, dict):
        for key, val in obj.items():
            if key == "instructions" and isinstance(val, list):
                new = []
                for inst in val:
                    if isinstance(inst, dict) and "sync_info" in inst:
                        new.extend(_split_instruction_waits(inst))
                    else:
                        _walk_split(inst)
                        new.append(inst)
                obj[key] = new
            else:
                _walk_split(val)
    elif isinstance(obj, list):
        for item in obj:
            _walk_split(item)


def _install_waitsplit():
    import concourse.bass as bass
    if getattr(bass.Bass, "_waitsplit_installed", False):
        return
    orig = bass.Bass.to_json_bytes

    def to_json_bytes(self, *a, **kw):
        d = _json.loads(orig(self, *a, **kw))
        _walk_split(d)
        return _json.dumps(d).encode()

    bass.Bass.to_json_bytes = to_json_bytes
    bass.Bass._waitsplit_installed = True


# ----------------------------------------------------------------------------
# Kernel builder
# ----------------------------------------------------------------------------

def _build(Bc=4, L=_L, D=_D, E=_E, LCHUNK=512, use_f32r=True):
    import concourse.bass as bass
    import concourse.mybir as mybir
    import concourse.tile as tile
    from concourse.masks import make_identity

    F32 = mybir.dt.float32
    F32R = mybir.dt.float32r
    AF = mybir.ActivationFunctionType
    X = mybir.AxisListType.X

    P = 128
    DT, ET, LT = D // P, E // P, L // P
    NCH = L // LCHUNK
    LCT = LCHUNK // P
    NE = (E + 511) // 512
    assert D % P == 0 and E % P == 0 and L % LCHUNK == 0 and LCHUNK % P == 0

    def r(ap):
        return ap.bitcast(F32R) if use_f32r else ap

    nc = bass.Bass("TRN2", target_bir_lowering=False, debug=False)
    enc = nc.dram_tensor("enc", [Bc, L, E], F32, kind="ExternalInput").ap()
    dec = nc.dram_tensor("dec", [Bc, D], F32, kind="ExternalInput").ap()
    W_s = nc.dram_tensor("W_s", [D, D], F32, kind="ExternalInput").ap()
    W_h = nc.dram_tensor("W_h", [D, E], F32, kind="ExternalInput").ap()
    v = nc.dram_tensor("v", [D], F32, kind="ExternalInput").ap()
    ctx_o = nc.dram_tensor("ctx", [Bc, E], F32, kind="ExternalOutput").ap()
    attn_o = nc.dram_tensor("attn", [Bc, L], F32, kind="ExternalOutput").ap()

    _cnt = [0]

    with tile.TileContext(nc) as tc:
        with ExitStack() as es:
            const_p = es.enter_context(tc.tile_pool(name="const", bufs=1))
            whT_p = es.enter_context(tc.tile_pool(name="whT", bufs=1))
            sT_p = es.enter_context(tc.tile_pool(name="sT", bufs=1))
            ps_t = es.enter_context(tc.tile_pool(name="ps_t", bufs=2, space="PSUM"))

            ident = const_p.tile([P, P], F32)
            make_identity(nc, ident[:])

            def evac(dst, src, rnd=False):
                if rnd and use_f32r:
                    dst = dst.bitcast(F32R)
                _cnt[0] += 1
                if _cnt[0] % 2 == 0:
                    nc.vector.tensor_copy(out=dst, in_=src)
                else:
                    nc.scalar.copy(out=dst, in_=src)

            # W_hT[t] = W_h[:, t*128:(t+1)*128].T  -> [128 e', D]
            whT = [whT_p.tile([P, D], F32, tag=f"whT{t}", name=f"whT{t}")
                   for t in range(ET)]
            with tc.tile_pool(name="wh_nat", bufs=2) as wh_nat_p:
                for a in range(DT):
                    wnat = wh_nat_p.tile([P, E], F32, tag="wnat")
                    nc.sync.dma_start(wnat[:], W_h[a * P:(a + 1) * P, :])
                    for t in range(ET):
                        ps = ps_t.tile([P, P], F32, tag="ps_t")
                        nc.tensor.transpose(ps[:], wnat[:, t * P:(t + 1) * P], ident[:])
                        evac(whT[t][:, a * P:(a + 1) * P], ps[:], rnd=True)

            # sT[J][j', b] = s_b[J*128+j'],  v_sb[:, J] = v[J*128:(J+1)*128]
            sT = [sT_p.tile([P, Bc], F32, tag=f"sT{J}", name=f"sT{J}")
                  for J in range(DT)]
            v_sb = sT_p.tile([P, DT], F32, tag="v_sb")
            with tc.tile_pool(name="setup_tmp", bufs=2) as tmp_p, \
                 tc.tile_pool(name="wsT", bufs=1) as wsT_p, \
                 tc.tile_pool(name="decT", bufs=1) as decT_p, \
                 tc.tile_pool(name="ps_s", bufs=2, space="PSUM") as ps_s:
                wsT = [wsT_p.tile([P, D], F32, tag=f"wsT{t}", name=f"wsT{t}")
                       for t in range(DT)]
                for a in range(DT):
                    wsnat = tmp_p.tile([P, D], F32, tag="wsnat")
                    nc.sync.dma_start(wsnat[:], W_s[a * P:(a + 1) * P, :])
                    for t in range(DT):
                        ps = ps_t.tile([P, P], F32, tag="ps_t")
                        nc.tensor.transpose(ps[:], wsnat[:, t * P:(t + 1) * P], ident[:])
                        evac(wsT[t][:, a * P:(a + 1) * P], ps[:])
                dec_sb = tmp_p.tile([Bc, D], F32, tag="dec_sb")
                nc.sync.dma_start(dec_sb[:], dec[:, :])
                decT = [decT_p.tile([P, Bc], F32, tag=f"decT{a}", name=f"decT{a}")
                        for a in range(DT)]
                for a in range(DT):
                    ps = ps_t.tile([P, P], F32, tag="ps_t")
                    nc.tensor.transpose(ps[:, :Bc], dec_sb[:, a * P:(a + 1) * P],
                                        ident[0:Bc, 0:Bc])
                    evac(decT[a][:], ps[:, :Bc])
                for J in range(DT):
                    ps = ps_s.tile([P, Bc], F32, tag="ps_s")
                    for a in range(DT):
                        nc.tensor.matmul(ps[:], wsT[a][:, J * P:(J + 1) * P],
                                         decT[a][:],
                                         start=(a == 0), stop=(a == DT - 1))
                    evac(sT[J][:], ps[:])
                v8 = tmp_p.tile([DT, P], F32, tag="v8")
                nc.sync.dma_start(v8[:], v.rearrange("(a b) -> a b", a=DT))
                ps = ps_t.tile([P, P], F32, tag="ps_t")
                nc.tensor.transpose(ps[:, :DT], v8[:], ident[0:DT, 0:DT])
                evac(v_sb[:], ps[:, :DT], rnd=True)

            enc_p = es.enter_context(tc.tile_pool(name="encp", bufs=5))
            encT_p = es.enter_context(tc.tile_pool(name="encT", bufs=2))
            t_p = es.enter_context(tc.tile_pool(name="tp", bufs=3))
            rows_p = es.enter_context(tc.tile_pool(name="rows", bufs=1))
            ps_h = es.enter_context(tc.tile_pool(name="ps_h", bufs=4, space="PSUM"))
            ps_sc = es.enter_context(tc.tile_pool(name="ps_sc", bufs=1, space="PSUM"))
            ps_cx = es.enter_context(tc.tile_pool(name="ps_cx", bufs=1, space="PSUM"))

            for b in range(Bc):
                # ---------------- pass 1: scores ----------------
                scores_row = rows_p.tile([1, L], F32, tag="scores_row")
                for c in range(NCH):
                    enc_nat = []
                    for k in range(LCT):
                        en = enc_p.tile([P, E], F32, tag="enc")
                        l0 = c * LCHUNK + k * P
                        nc.sync.dma_start(en[:], enc[b, l0:l0 + P, :])
                        enc_nat.append(en)
                    encT = [encT_p.tile([P, LCHUNK], F32, tag=f"encT{t}",
                                        name=f"encT{t}") for t in range(ET)]
                    for t in range(ET):
                        for k in range(LCT):
                            ps = ps_t.tile([P, P], F32, tag="ps_t")
                            nc.tensor.transpose(ps[:],
                                                enc_nat[k][:, t * P:(t + 1) * P],
                                                ident[:])
                            evac(encT[t][:, k * P:(k + 1) * P], ps[:], rnd=True)
                    psc = ps_sc.tile([1, LCHUNK], F32, tag="ps_sc")
                    for j in range(DT):
                        ph = ps_h.tile([P, LCHUNK], F32, tag="ps_h")
                        for t in range(ET):
                            nc.tensor.matmul(ph[:], r(whT[t][:, j * P:(j + 1) * P]),
                                             r(encT[t][:]),
                                             start=(t == 0), stop=(t == ET - 1))
                        tj = t_p.tile([P, LCHUNK], F32, tag="tj")
                        nc.scalar.activation(r(tj[:]), ph[:], AF.Tanh,
                                             bias=sT[j][:, b:b + 1])
                        nc.tensor.matmul(psc[:], r(v_sb[:, j:j + 1]), r(tj[:]),
                                         start=(j == 0), stop=(j == DT - 1))
                    nc.vector.tensor_copy(
                        out=scores_row[:, c * LCHUNK:(c + 1) * LCHUNK], in_=psc[:])

                # ---------------- softmax ----------------
                m = rows_p.tile([1, 1], F32, tag="m")
                nc.vector.reduce_max(m[:], scores_row[:], axis=X)
                nm = rows_p.tile([1, 1], F32, tag="nm")
                nc.vector.tensor_scalar_mul(nm[:], m[:], -1.0)
                erow = rows_p.tile([1, L], F32, tag="erow")
                nc.scalar.activation(erow[:], scores_row[:], AF.Exp, bias=nm[:])
                z = rows_p.tile([1, 1], F32, tag="z")
                nc.vector.reduce_sum(z[:], erow[:], axis=X)
                rz = rows_p.tile([1, 1], F32, tag="rz")
                nc.vector.reciprocal(rz[:], z[:])
                nc.vector.tensor_scalar_mul(erow[:], erow[:], rz[:])
                nc.sync.dma_start(attn_o[b:b + 1, :], erow[:])

                # attn as [l, 1] tiles: [1, L] -> [LT, 128] -> T -> [128, LT]
                attn_rect = rows_p.tile([LT, P], F32, tag="attn_rect")
                nc.sync.dma_start(attn_rect[:], erow[:])
                ps = ps_t.tile([P, P], F32, tag="ps_t")
                nc.tensor.transpose(ps[:, :LT], attn_rect[:], ident[0:LT, 0:LT])
                attn_t = rows_p.tile([P, LT], F32, tag="attn_t")
                evac(attn_t[:], ps[:, :LT], rnd=True)

                # ---------------- pass 2: ctx ----------------
                ctx_row = rows_p.tile([1, E], F32, tag="ctx_row")
                BS = 4
                for B0 in range(0, LT, BS):
                    blk = []
                    for i in range(B0, min(B0 + BS, LT)):
                        en = enc_p.tile([P, E], F32, tag="enc")
                        nc.sync.dma_start(r(en[:]), r(enc[b, i * P:(i + 1) * P, :]))
                        blk.append((i, en))
                    for n in range(NE):
                        n0 = n * 512
                        n1 = min(n0 + 512, E)
                        pc = ps_cx.tile([1, 512], F32, tag="ps_cx")
                        for zix, (i, en) in enumerate(blk):
                            nc.tensor.matmul(pc[:, :n1 - n0], r(attn_t[:, i:i + 1]),
                                             r(en[:, n0:n1]),
                                             start=(zix == 0),
                                             stop=(zix == len(blk) - 1))
                        if B0 == 0:
                            nc.vector.tensor_copy(out=ctx_row[:, n0:n1],
                                                  in_=pc[:, :n1 - n0])
                        else:
                            nc.vector.tensor_add(out=ctx_row[:, n0:n1],
                                                 in0=ctx_row[:, n0:n1],
                                                 in1=pc[:, :n1 - n0])
                nc.sync.dma_start(ctx_o[b:b + 1, :], ctx_row[:])

    return nc


_cache = {}


def _get_nc():
    if "nc" not in _cache:
        _install_waitsplit()
        _cache["nc"] = _build()
    return _cache["nc"]


def run(inputs, trace=False, **run_kwargs):
    """Run on 8 NeuronCores; returns (ctx, attn, BassKernelResults)."""
    from concourse.bass_utils import run_bass_kernel_spmd

    nc = _get_nc()
    enc = np.ascontiguousarray(np.asarray(inputs["enc_outputs"], dtype=np.float32))
    dec = np.ascontiguousarray(np.asarray(inputs["dec_hidden"], dtype=np.float32))
    W_s = np.ascontiguousarray(np.asarray(inputs["W_s"], dtype=np.float32))
    W_h = np.ascontiguousarray(np.asarray(inputs["W_h"], dtype=np.float32))
    v = np.ascontiguousarray(np.asarray(inputs["v"], dtype=np.float32))
    B = enc.shape[0]
    Bc = B // _NCORES
    in_maps = []
    for i in range(_NCORES):
        in_maps.append({
            "enc": enc[i * Bc:(i + 1) * Bc],
            "dec": dec[i * Bc:(i + 1) * Bc],
            "W_s": W_s,
            "W_h": W_h,
            "v": v,
        })
    res = run_bass_kernel_spmd(nc, in_maps, core_ids=list(range(_NCORES)),
                               trace=trace, **run_kwargs)
    ctx = np.concatenate([res.results[i]["ctx"] for i in range(_NCORES)], axis=0)
    attn = np.concatenate([res.results[i]["attn"] for i in range(_NCORES)], axis=0)
    return ctx, attn, res


def kernel(**inputs):
    ctx, attn, _ = run(inputs, trace=False)
    return ctx, attn
